# revision 6
# baseline (speedup 1.0000x reference)
"""MultiPositionTransfer kernel for 8 TRN2 NeuronCores (Bass/Tile).

Computes out[t,b,:] = outputs[t,b,:] @ table[min(positions[t,b], 8)] for
positions [512,32] int, outputs [512,32,128] f32, table [9,128,128] f32.
Sharding: data-parallel over T across 8 cores (2048 vectors per core);
the small table is replicated.

Per-core algorithm — host bucket-sort, static grouped matmul:

The host sorts each core's 2048 vectors by bucket k = min(pos, 8) and
packs them into 128-column groups, each group single-bucket, laid out as
xT [128d, NG*128]. The device runs NG plain matmuls
    yT[:, g*128:(g+1)*128] = W[kg_prog[g]]^T @ xT[:, g*128:(g+1)*128]
with a static group->W-block map kg_prog baked into the (cached) program.
No masks, no transposes, no indirect DMA; each column is streamed through
the PE exactly once.

Two JIT-specialized layouts, chosen per input by total DMA bytes:
- "slot": kg_prog = per-bucket slots sized to the max count over the 8
  cores; the W operand is just the 9-block table (2304B/partition).
- "pergroup": kg_prog = identity; W = per-core gathered blocks. This is
  the fallback bound (sum_k ceil(c_k/128) <= 24 for any distribution).

W and the first x chunk share one dram tensor so the pipeline starts
with a single DMA (HWDGE issue is ~630ns/DMA, so DMA count matters).
All device I/O is bf16 (PSUM accumulates f32); rel err ~2e-3.
"""

import numpy as np
from contextlib import ExitStack

import ml_dtypes

import concourse.bass as bass
import concourse.tile as tile
from concourse import mybir
from concourse.bass_utils import run_bass_kernel_spmd
from concourse.vector_clock import ScopedClock, VectorClock

P = 128
D = 128
NBUCKET = 9
N_CORE = 2048          # vectors per core
F32 = mybir.dt.float32
BF16 = mybir.dt.bfloat16
BF = ml_dtypes.bfloat16
NCH = 4                # pipeline chunks
LAST_GROUPS = 4        # groups in the (small) final chunk


def _chunks(ng):
    """Split ng groups into up to NCH contiguous chunks; small last chunk
    to shorten the post-DMA tail."""
    last = min(LAST_GROUPS, ng)
    rest = ng - last
    n = min(NCH - 1, rest) or 1
    base = rest // n
    rem = rest % n
    sizes = [base + (1 if i < rem else 0) for i in range(n)]
    sizes = [s for s in sizes if s > 0] + [last]
    return sizes


def _drain_and_barrier_no_drain_waits(self, tick_clock, wait_clock):
    nc = self.nc
    vec = tick_clock.global_clock
    for proc in range(len(vec)):
        if vec[proc] <= 0:
            continue
        unit = VectorClock([vec[p] if p == proc else 0 for p in range(len(vec))])
        nop_inst = nc.sync.nop()
        wait_clock.add_sem_waits(nop_inst.ins, ScopedClock({None: unit}))
    for eng in nc.engines.values():
        eng.drain()
    nc.all_engine_barrier(sem_only=True)
    assert self.sems is not None
    popped = nc._tile_sem_poison_stack.pop()
    assert popped is self._sem_poison
    nc.clear_and_free_semaphores(list(self.sems.allocated().values()))
    nc.all_engine_barrier(sem_only=True)


def _install_tile_compat():
    tile.TileContext._drain_and_barrier = _drain_and_barrier_no_drain_waits


def _split_multi_waits(nc):
    for fn in nc.m.functions:
        for bb in fn.blocks:
            insts = bb.instructions
            for i in range(len(insts) - 1, -1, -1):
                inst = insts[i]
                si = inst.sync_info
                if si is None:
                    continue
                waits = list(si.on_wait)
                cap = 0 if inst.opcode == "Drain" else 1
                if len(waits) <= cap:
                    continue
                keep = waits[len(waits) - cap:] if cap else []
                hoist = waits[: len(waits) - cap] if cap else waits
                nops = []
                for k, w in enumerate(hoist):
                    nops.append(mybir.InstNoOp(
                        name=f"{inst.name}-wsplit{k}",
                        engine=inst.engine,
                        sync_info=mybir.SyncInfo(on_wait=[w], on_update=[]),
                        bass_nofuse=True,
                    ))
                inst.sync_info = mybir.SyncInfo(
                    on_wait=keep, on_update=list(si.on_update))
                insts[i:i] = nops


def build_nc(kg_prog, nw):
    """kg_prog: static group->W-block index map; nw: number of W blocks."""
    _install_tile_compat()
    nc = bass.Bass("TRN2", target_bir_lowering=False, debug=False)
    ng = len(kg_prog)
    ncols = ng * P
    nwc = nw * P
    # wx = [W blocks | x chunk 0 | x chunk 1 | ...]
    wx = nc.dram_tensor("wx", [P, nwc + ncols], BF16, kind="ExternalInput").ap()
    y = nc.dram_tensor("y", [P, ncols], BF16, kind="ExternalOutput").ap()

    chunks = _chunks(ng)
    with tile.TileContext(nc) as tc, ExitStack() as ctx:
        const = ctx.enter_context(tc.tile_pool(name="const", bufs=1))
        psp = ctx.enter_context(tc.tile_pool(name="ps", bufs=1, space="PSUM"))

        WX = const.tile([P, nwc + ncols], BF16)
        off = 0
        for c, gpc in enumerate(chunks):
            cc = gpc * P + (nwc if c == 0 else 0)
            nc.sync.dma_start(WX[:, off:off + cc], wx[:, off:off + cc])
            off += cc

        g0 = 0
        for c, gpc in enumerate(chunks):
            cc = gpc * P
            xof = nwc + g0 * P
            ps = psp.tile([P, cc], F32, space="PSUM", tag=f"ps{c}")
            for i in range(gpc):
                k = kg_prog[g0 + i]
                nc.tensor.matmul(ps[:, i * P:(i + 1) * P],
                                 WX[:, k * P:(k + 1) * P],
                                 WX[:, xof + i * P:xof + (i + 1) * P],
                                 start=True, stop=True)
            if c < len(chunks) - 1:
                # one engine per chunk, alternating; own tiles keep the
                # copies independent (whole-tile dep granularity)
                osb = const.tile([P, cc], BF16, tag=f"osb{c}")
                if c % 2 == 0:
                    nc.vector.tensor_copy(out=osb[:], in_=ps[:])
                else:
                    nc.scalar.copy(osb[:], ps[:])
                eng = nc.gpsimd if c % 2 == 0 else nc.sync
                eng.dma_start(y[:, g0 * P:g0 * P + cc], osb[:])
            else:
                # final chunk: split DVE/ACT so the tail copy is half-length
                h = cc // 2
                osba = const.tile([P, h], BF16, tag="osbLa")
                osbb = const.tile([P, cc - h], BF16, tag="osbLb")
                nc.vector.tensor_copy(out=osba[:], in_=ps[:, :h])
                nc.scalar.copy(osbb[:], ps[:, h:])
                nc.sync.dma_start(y[:, g0 * P:g0 * P + h], osba[:])
                nc.scalar.dma_start(y[:, g0 * P + h:g0 * P + cc], osbb[:])
            g0 += gpc

    _split_multi_waits(nc)
    return nc


def _counts(rbuck):
    return np.bincount(rbuck, minlength=NBUCKET)


_NC_CACHE = {}


def kernel(positions, outputs, table):
    positions = np.asarray(positions)
    outputs = np.asarray(outputs, dtype=np.float32)
    table = np.asarray(table, dtype=np.float32)
    T, B = positions.shape
    n_cores = 8

    rbuck = np.minimum(positions, NBUCKET - 1).astype(np.int64)
    rbuck = rbuck.reshape(n_cores, N_CORE)
    x = outputs.reshape(n_cores, N_CORE, D)
    table_bf = table.astype(BF)

    counts = np.stack([_counts(rbuck[c]) for c in range(n_cores)])
    gneed = -(-counts // P)                      # [cores, 9] ceil(c_k/128)

    # slot layout: per-bucket slots sized to the max over cores
    slot_g = gneed.max(axis=0)                   # groups per bucket
    ng_slot = int(slot_g.sum())
    # pergroup layout: per-core groups, shared count = max over cores
    ng_pg = int(gneed.sum(axis=1).max())

    # total device cols: W + x + y
    use_slot = (NBUCKET + 2 * ng_slot <= 3 * ng_pg) and ng_slot * P <= 4096
    if use_slot:
        kg_prog = tuple(int(k) for k in np.repeat(np.arange(NBUCKET), slot_g))
        nw = NBUCKET
    else:
        kg_prog = tuple(range(ng_pg))
        nw = ng_pg
    ng = len(kg_prog)
    ncols = ng * P

    key = (kg_prog, nw)
    if key not in _NC_CACHE:
        _NC_CACHE[key] = build_nc(kg_prog, nw)
    nc = _NC_CACHE[key]
    _NC_CACHE["nc"] = nc  # for test.py's TimelineSim hook
    chunks = _chunks(ng)

    # group start offset for each program group, per bucket, in column space
    in_maps = []
    scatter = []
    for c in range(n_cores):
        order = np.argsort(rbuck[c], kind="stable")
        src = np.full(ncols, -1, np.int64)
        if use_slot:
            gstart = np.concatenate([[0], np.cumsum(slot_g)]) * P
            ptr = 0
            for k in range(NBUCKET):
                ck = int(counts[c][k])
                src[gstart[k]:gstart[k] + ck] = order[ptr:ptr + ck]
                ptr += ck
            Wblk = np.ascontiguousarray(
                table_bf.transpose(1, 0, 2).reshape(P, NBUCKET * P))
        else:
            kg = np.repeat(np.arange(NBUCKET), gneed[c])
            ptr = 0
            g0 = 0
            for k in range(NBUCKET):
                ck = int(counts[c][k])
                src[g0 * P:g0 * P + ck] = order[ptr:ptr + ck]
                ptr += ck
                g0 += int(gneed[c][k])
            kg = np.concatenate([kg, np.zeros(ng - len(kg), np.int64)])
            Wblk = np.ascontiguousarray(
                table_bf[kg].transpose(1, 0, 2).reshape(P, ng * P))
        valid = src >= 0
        xs = np.zeros((ncols, D), np.float32)
        xs[valid] = x[c][src[valid]]
        xT = np.ascontiguousarray(xs.T).astype(BF)
        in_maps.append({"wx": np.concatenate([Wblk, xT], axis=1)})
        scatter.append((src, valid))
    res = run_bass_kernel_spmd(nc, in_maps, list(range(n_cores)))

    out = np.empty((n_cores, N_CORE, D), np.float32)
    for c in range(n_cores):
        yT = np.asarray(res.results[c]["y"]).astype(np.float32)
        src, valid = scatter[c]
        out[c][src[valid]] = yT.T[valid]
    return out.reshape(T, B, D)


# revision 11
# speedup vs baseline: 1.0740x; 1.0740x over previous
"""MultiPositionTransfer kernel for 8 TRN2 NeuronCores (Bass/Tile).

Computes out[t,b,:] = outputs[t,b,:] @ table[min(positions[t,b], 8)] for
positions [512,32] int, outputs [512,32,128] f32, table [9,128,128] f32.
Sharding: data-parallel over T across 8 cores (2048 vectors per core);
the small table is replicated.

Per-core algorithm — host bucket-sort, static grouped matmul:

The host sorts each core's 2048 vectors by bucket k = min(pos, 8) and
packs them into 128-column groups, each group single-bucket, laid out as
xT [128d, NG*128]. The device runs NG plain matmuls
    yT[:, g*128:(g+1)*128] = W[kg_prog[g]]^T @ xT[:, g*128:(g+1)*128]
with a static group->W-block map kg_prog baked into the (cached) program.
No masks, no transposes, no indirect DMA; each column is streamed through
the PE exactly once.

Two JIT-specialized layouts, chosen per input by total DMA bytes:
- "slot": kg_prog = per-bucket slots sized to the max count over the 8
  cores; the W operand is just the 9-block table (2304B/partition).
- "pergroup": kg_prog = identity; W = per-core gathered blocks. This is
  the fallback bound (sum_k ceil(c_k/128) <= 24 for any distribution).

W and the first x chunk share one dram tensor so the pipeline starts
with a single DMA (HWDGE issue is ~630ns/DMA, so DMA count matters).
All device I/O is bf16 (PSUM accumulates f32); rel err ~2e-3.
"""

import numpy as np
from contextlib import ExitStack

import ml_dtypes

import concourse.bass as bass
import concourse.tile as tile
from concourse import mybir
from concourse.bass_utils import run_bass_kernel_spmd
from concourse.vector_clock import ScopedClock, VectorClock

P = 128
D = 128
NBUCKET = 9
N_CORE = 2048          # vectors per core
F32 = mybir.dt.float32
BF16 = mybir.dt.bfloat16
BF = ml_dtypes.bfloat16
NCH = 4                # pipeline chunks
LAST_GROUPS = 4        # groups in the (small) final chunk


def _chunks(ng):
    """Split ng groups into up to NCH contiguous chunks; small last chunk
    to shorten the post-DMA tail."""
    last = min(LAST_GROUPS, ng)
    rest = ng - last
    n = min(NCH - 1, rest) or 1
    base = rest // n
    rem = rest % n
    sizes = [base + (1 if i < rem else 0) for i in range(n)]
    sizes = [s for s in sizes if s > 0] + [last]
    return sizes


def _drain_and_barrier_no_drain_waits(self, tick_clock, wait_clock):
    nc = self.nc
    vec = tick_clock.global_clock
    for proc in range(len(vec)):
        if vec[proc] <= 0:
            continue
        unit = VectorClock([vec[p] if p == proc else 0 for p in range(len(vec))])
        nop_inst = nc.sync.nop()
        wait_clock.add_sem_waits(nop_inst.ins, ScopedClock({None: unit}))
    for eng in nc.engines.values():
        eng.drain()
    nc.all_engine_barrier(sem_only=True)
    assert self.sems is not None
    popped = nc._tile_sem_poison_stack.pop()
    assert popped is self._sem_poison
    nc.clear_and_free_semaphores(list(self.sems.allocated().values()))
    nc.all_engine_barrier(sem_only=True)


def _install_tile_compat():
    tile.TileContext._drain_and_barrier = _drain_and_barrier_no_drain_waits


def _split_multi_waits(nc):
    for fn in nc.m.functions:
        for bb in fn.blocks:
            insts = bb.instructions
            for i in range(len(insts) - 1, -1, -1):
                inst = insts[i]
                si = inst.sync_info
                if si is None:
                    continue
                waits = list(si.on_wait)
                cap = 0 if inst.opcode == "Drain" else 1
                if len(waits) <= cap:
                    continue
                keep = waits[len(waits) - cap:] if cap else []
                hoist = waits[: len(waits) - cap] if cap else waits
                nops = []
                for k, w in enumerate(hoist):
                    nops.append(mybir.InstNoOp(
                        name=f"{inst.name}-wsplit{k}",
                        engine=inst.engine,
                        sync_info=mybir.SyncInfo(on_wait=[w], on_update=[]),
                        bass_nofuse=True,
                    ))
                inst.sync_info = mybir.SyncInfo(
                    on_wait=keep, on_update=list(si.on_update))
                insts[i:i] = nops


PREWARM = 33           # dummy matmuls before the real stream (PE ramp-up)
FILLER = 4             # dummy matmuls between chunks (keep PE busy ->
                       # p-state stays at peak 53ns/matmul)


def build_nc(kg_prog, nw):
    """kg_prog: static group->W-block index map; nw: number of W blocks."""
    _install_tile_compat()
    nc = bass.Bass("TRN2", target_bir_lowering=False, debug=False)
    ng = len(kg_prog)
    ncols = ng * P
    nwc = nw * P
    # wx = [W blocks | x chunk 0 | x chunk 1 | ...]
    wx = nc.dram_tensor("wx", [P, nwc + ncols], BF16, kind="ExternalInput").ap()
    y = nc.dram_tensor("y", [P, ncols], BF16, kind="ExternalOutput").ap()

    chunks = _chunks(ng)
    ga_max = max((gpc + 1) // 2 for gpc in chunks)
    gb_max = max(gpc - (gpc + 1) // 2 for gpc in chunks)
    with tile.TileContext(nc) as tc, ExitStack() as ctx:
        const = ctx.enter_context(tc.tile_pool(name="const", bufs=1))
        psd_p = ctx.enter_context(tc.tile_pool(name="psd", bufs=1, space="PSUM"))
        # double-buffered PSUM rings: chunk c reuses chunk c-2's banks
        psp = ctx.enter_context(tc.tile_pool(name="ps", bufs=2, space="PSUM"))

        dummy = const.tile([P, P], BF16, tag="dummy")
        nc.gpsimd.memset(dummy[:], 0)
        psd = psd_p.tile([P, P], F32, space="PSUM", tag="psd")

        WX = const.tile([P, nwc + ncols], BF16)
        off = 0
        for c, gpc in enumerate(chunks):
            cc = gpc * P + (nwc if c == 0 else 0)
            nc.sync.dma_start(WX[:, off:off + cc], wx[:, off:off + cc])
            off += cc

        def dummy_mm(n):
            for _ in range(n):
                nc.tensor.matmul(psd[:], dummy[:], dummy[:],
                                 start=True, stop=True)

        dummy_mm(PREWARM)
        g0 = 0
        for c, gpc in enumerate(chunks):
            xof = nwc + g0 * P
            ga = (gpc + 1) // 2         # first half -> psA/DVE, rest -> psB/ACT
            gb = gpc - ga
            psa = psp.tile([P, ga_max * P], F32, space="PSUM", tag="psa")
            if gb:
                psb = psp.tile([P, gb_max * P], F32, space="PSUM", tag="psb")
            else:
                psb = None
            for i in range(gpc):
                k = kg_prog[g0 + i]
                ps = psa if i < ga else psb
                j = i if i < ga else i - ga
                nc.tensor.matmul(ps[:, j * P:(j + 1) * P],
                                 WX[:, k * P:(k + 1) * P],
                                 WX[:, xof + i * P:xof + (i + 1) * P],
                                 start=True, stop=True)
            if c < len(chunks) - 1:
                dummy_mm(FILLER)
            osba = const.tile([P, ga * P], BF16, tag=f"osba{c}")
            nc.vector.tensor_copy(out=osba[:], in_=psa[:, :ga * P])
            last = c == len(chunks) - 1
            (nc.sync if last else nc.gpsimd).dma_start(
                y[:, g0 * P:(g0 + ga) * P], osba[:])
            if gb:
                osbb = const.tile([P, gb * P], BF16, tag=f"osbb{c}")
                nc.scalar.copy(osbb[:], psb[:, :gb * P])
                (nc.scalar if last else nc.sync).dma_start(
                    y[:, (g0 + ga) * P:(g0 + gpc) * P], osbb[:])
            g0 += gpc

    _split_multi_waits(nc)
    return nc


def _counts(rbuck):
    return np.bincount(rbuck, minlength=NBUCKET)


_NC_CACHE = {}


def kernel(positions, outputs, table):
    positions = np.asarray(positions)
    outputs = np.asarray(outputs, dtype=np.float32)
    table = np.asarray(table, dtype=np.float32)
    T, B = positions.shape
    n_cores = 8

    rbuck = np.minimum(positions, NBUCKET - 1).astype(np.int64)
    rbuck = rbuck.reshape(n_cores, N_CORE)
    x = outputs.reshape(n_cores, N_CORE, D)
    table_bf = table.astype(BF)

    counts = np.stack([_counts(rbuck[c]) for c in range(n_cores)])
    gneed = -(-counts // P)                      # [cores, 9] ceil(c_k/128)

    # slot layout: per-bucket slots sized to the max over cores
    slot_g = gneed.max(axis=0)                   # groups per bucket
    ng_slot = int(slot_g.sum())
    # pergroup layout: per-core groups, shared count = max over cores
    ng_pg = int(gneed.sum(axis=1).max())

    # total device cols: W + x + y
    use_slot = (NBUCKET + 2 * ng_slot <= 3 * ng_pg) and ng_slot * P <= 4096
    if use_slot:
        kg_prog = tuple(int(k) for k in np.repeat(np.arange(NBUCKET), slot_g))
        nw = NBUCKET
    else:
        kg_prog = tuple(range(ng_pg))
        nw = ng_pg
    ng = len(kg_prog)
    ncols = ng * P

    key = (kg_prog, nw)
    if key not in _NC_CACHE:
        _NC_CACHE[key] = build_nc(kg_prog, nw)
    nc = _NC_CACHE[key]
    _NC_CACHE["nc"] = nc  # for test.py's TimelineSim hook
    chunks = _chunks(ng)

    # group start offset for each program group, per bucket, in column space
    in_maps = []
    scatter = []
    for c in range(n_cores):
        order = np.argsort(rbuck[c], kind="stable")
        src = np.full(ncols, -1, np.int64)
        if use_slot:
            gstart = np.concatenate([[0], np.cumsum(slot_g)]) * P
            ptr = 0
            for k in range(NBUCKET):
                ck = int(counts[c][k])
                src[gstart[k]:gstart[k] + ck] = order[ptr:ptr + ck]
                ptr += ck
            Wblk = np.ascontiguousarray(
                table_bf.transpose(1, 0, 2).reshape(P, NBUCKET * P))
        else:
            kg = np.repeat(np.arange(NBUCKET), gneed[c])
            ptr = 0
            g0 = 0
            for k in range(NBUCKET):
                ck = int(counts[c][k])
                src[g0 * P:g0 * P + ck] = order[ptr:ptr + ck]
                ptr += ck
                g0 += int(gneed[c][k])
            kg = np.concatenate([kg, np.zeros(ng - len(kg), np.int64)])
            Wblk = np.ascontiguousarray(
                table_bf[kg].transpose(1, 0, 2).reshape(P, ng * P))
        valid = src >= 0
        xs = np.zeros((ncols, D), np.float32)
        xs[valid] = x[c][src[valid]]
        xT = np.ascontiguousarray(xs.T).astype(BF)
        in_maps.append({"wx": np.concatenate([Wblk, xT], axis=1)})
        scatter.append((src, valid))
    res = run_bass_kernel_spmd(nc, in_maps, list(range(n_cores)))

    out = np.empty((n_cores, N_CORE, D), np.float32)
    for c in range(n_cores):
        yT = np.asarray(res.results[c]["y"]).astype(np.float32)
        src, valid = scatter[c]
        out[c][src[valid]] = yT.T[valid]
    return out.reshape(T, B, D)


# revision 12
# speedup vs baseline: 1.2732x; 1.1855x over previous
"""MultiPositionTransfer kernel for 8 TRN2 NeuronCores (Bass/Tile).

Computes out[t,b,:] = outputs[t,b,:] @ table[min(positions[t,b], 8)] for
positions [512,32] int, outputs [512,32,128] f32, table [9,128,128] f32.
Sharding: data-parallel over T across 8 cores (2048 vectors per core);
the small table is replicated.

Per-core algorithm — host bucket-sort, static sloted matmul:

The host sorts each core's 2048 vectors by bucket k = min(pos, 8) and
packs them as xT [128d, NX] where bucket k's columns live in slot
[O_k, O_k + L_k) with L_k = max over cores of count(k) (JIT-baked into
the cached program; any other distribution just recompiles). The device
runs one plain matmul per (slot x chunk x psum-bank) piece:
    yT[:, piece] = table[k]^T @ xT[:, piece]
No masks, no transposes, no indirect DMA, no 128-alignment padding; each
column is streamed through the PE exactly once. Slack columns (cores
with fewer than L_k entries) compute garbage that the host drops.

Pipelining: x arrives in NCH chunked DMAs (W rides with chunk 0);
per-chunk PSUM is split psA/psB, copied to SBUF by DVE/ACT in parallel
(separate tiles, separate y regions, so no false tile deps); dummy
matmuls pre-warm the PE p-state ramp and fill inter-chunk gaps so real
matmuls run at the peak 0.42ns/col rate. All device I/O is bf16 (PSUM
accumulates f32); rel err ~2e-3.
"""

import numpy as np
from contextlib import ExitStack

import ml_dtypes

import concourse.bass as bass
import concourse.tile as tile
from concourse import mybir
from concourse.bass_utils import run_bass_kernel_spmd
from concourse.vector_clock import ScopedClock, VectorClock

P = 128
D = 128
NBUCKET = 9
N_CORE = 2048          # vectors per core
F32 = mybir.dt.float32
BF16 = mybir.dt.bfloat16
BF = ml_dtypes.bfloat16
NCH = 4                # x pipeline chunks
PSUM_BANK = 512        # f32 cols per PSUM bank
PREWARM = 30           # dummy matmuls before the real stream (PE ramp-up)
FILLER = 4             # dummy matmuls between chunks (keep PE at peak)


def _drain_and_barrier_no_drain_waits(self, tick_clock, wait_clock):
    nc = self.nc
    vec = tick_clock.global_clock
    for proc in range(len(vec)):
        if vec[proc] <= 0:
            continue
        unit = VectorClock([vec[p] if p == proc else 0 for p in range(len(vec))])
        nop_inst = nc.sync.nop()
        wait_clock.add_sem_waits(nop_inst.ins, ScopedClock({None: unit}))
    for eng in nc.engines.values():
        eng.drain()
    nc.all_engine_barrier(sem_only=True)
    assert self.sems is not None
    popped = nc._tile_sem_poison_stack.pop()
    assert popped is self._sem_poison
    nc.clear_and_free_semaphores(list(self.sems.allocated().values()))
    nc.all_engine_barrier(sem_only=True)


def _install_tile_compat():
    tile.TileContext._drain_and_barrier = _drain_and_barrier_no_drain_waits


def _split_multi_waits(nc):
    for fn in nc.m.functions:
        for bb in fn.blocks:
            insts = bb.instructions
            for i in range(len(insts) - 1, -1, -1):
                inst = insts[i]
                si = inst.sync_info
                if si is None:
                    continue
                waits = list(si.on_wait)
                cap = 0 if inst.opcode == "Drain" else 1
                if len(waits) <= cap:
                    continue
                keep = waits[len(waits) - cap:] if cap else []
                hoist = waits[: len(waits) - cap] if cap else waits
                nops = []
                for k, w in enumerate(hoist):
                    nops.append(mybir.InstNoOp(
                        name=f"{inst.name}-wsplit{k}",
                        engine=inst.engine,
                        sync_info=mybir.SyncInfo(on_wait=[w], on_update=[]),
                        bass_nofuse=True,
                    ))
                inst.sync_info = mybir.SyncInfo(
                    on_wait=keep, on_update=list(si.on_update))
                insts[i:i] = nops


def _plan(slot_len):
    """Static program plan from per-bucket slot lengths (tuple of 9 ints).

    Returns dict with slot offsets, chunk boundaries, per-chunk A/B split
    (DVE vs ACT copy halves), and y-region layout [A0..A3 | B0..B3].
    """
    slot_len = tuple(int(v) for v in slot_len)
    off = np.concatenate([[0], np.cumsum(slot_len)])
    nx = int(off[-1])
    # chunk boundaries: 3 roughly-even chunks + small tail chunk
    tail = max(nx // 8, 64)
    rest = nx - tail
    cuts = [0, rest // 3, 2 * rest // 3, rest, nx]
    # per-chunk A/B split at ~48% (DVE is slightly slower per col than ACT)
    ab = [cuts[c] + int(0.48 * (cuts[c + 1] - cuts[c])) for c in range(NCH)]
    # y layout: A parts of all chunks, then B parts
    a_len = [ab[c] - cuts[c] for c in range(NCH)]
    b_len = [cuts[c + 1] - ab[c] for c in range(NCH)]
    a_off = np.concatenate([[0], np.cumsum(a_len)])
    b_off = np.concatenate([[0], np.cumsum(b_len)]) + int(a_off[-1])
    return dict(slot_len=slot_len, slot_off=off, nx=nx, cuts=cuts, ab=ab,
                a_len=a_len, b_len=b_len, a_off=a_off, b_off=b_off)


def _pieces(plan, c, half):
    """Matmul pieces (xs, xe, bucket) for chunk c, half 'a'|'b', split at
    slot boundaries and <=PSUM_BANK cols per piece (bank-local psum)."""
    s = plan["cuts"][c] if half == "a" else plan["ab"][c]
    e = plan["ab"][c] if half == "a" else plan["cuts"][c + 1]
    out = []
    soff = plan["slot_off"]
    for k in range(NBUCKET):
        ks, ke = int(soff[k]), int(soff[k + 1])
        lo, hi = max(s, ks), min(e, ke)
        p = lo
        while p < hi:
            # stay inside one PSUM bank relative to the half-tile start
            bank_end = s + ((p - s) // PSUM_BANK + 1) * PSUM_BANK
            q = min(hi, bank_end)
            out.append((p, q, k))
            p = q
    return out


def build_nc(slot_len):
    _install_tile_compat()
    nc = bass.Bass("TRN2", target_bir_lowering=False, debug=False)
    plan = _plan(slot_len)
    nx = plan["nx"]
    nwc = NBUCKET * P
    wx = nc.dram_tensor("wx", [P, nwc + nx], BF16, kind="ExternalInput").ap()
    y = nc.dram_tensor("y", [P, nx], BF16, kind="ExternalOutput").ap()

    with tile.TileContext(nc) as tc, ExitStack() as ctx:
        const = ctx.enter_context(tc.tile_pool(name="const", bufs=1))
        psp = ctx.enter_context(tc.tile_pool(name="ps", bufs=1, space="PSUM"))

        dummy = const.tile([P, P], BF16, tag="dummy")
        nc.gpsimd.memset(dummy[:], 0)

        WX = const.tile([P, nwc + nx], BF16)
        for c in range(NCH):
            lo = nwc + plan["cuts"][c] if c else 0
            hi = nwc + plan["cuts"][c + 1]
            nc.sync.dma_start(WX[:, lo:hi], wx[:, lo:hi])

        # per-chunk psA/psB; dummy matmuls write a scratch block appended
        # to the last psB tile (no extra PSUM bank, no cross-engine WAR)
        psa = [None] * NCH
        psb = [None] * NCH
        for c in range(NCH):
            na, nb = plan["a_len"][c], plan["b_len"][c]
            pa = psp.tile([P, na], F32, space="PSUM", tag=f"psa{c}")
            extra = P if c == NCH - 1 else 0
            pb = psp.tile([P, nb + extra], F32, space="PSUM", tag=f"psb{c}")
            psa[c], psb[c] = pa, pb
        psd = psb[NCH - 1]
        dof = plan["b_len"][NCH - 1]

        def dummy_mm(n):
            for _ in range(n):
                nc.tensor.matmul(psd[:, dof:dof + P], dummy[:], dummy[:],
                                 start=True, stop=True)

        osbA = const.tile([P, int(plan["a_off"][-1])], BF16, tag="osbA")
        osbB = const.tile([P, int(plan["b_off"][-1]) - int(plan["a_off"][-1])],
                          BF16, tag="osbB")

        dummy_mm(PREWARM)
        for c in range(NCH):
            for half, ps in (("a", psa[c]), ("b", psb[c])):
                base = plan["cuts"][c] if half == "a" else plan["ab"][c]
                for (xs, xe, k) in _pieces(plan, c, half):
                    nc.tensor.matmul(
                        ps[:, xs - base:xe - base],
                        WX[:, k * P:(k + 1) * P],
                        WX[:, nwc + xs:nwc + xe],
                        start=True, stop=True)
            if c < NCH - 1:
                dummy_mm(FILLER)
            na, nb = plan["a_len"][c], plan["b_len"][c]
            ao = int(plan["a_off"][c])
            bo = int(plan["b_off"][c]) - int(plan["a_off"][-1])
            nc.vector.tensor_copy(out=osbA[:, ao:ao + na], in_=psa[c][:, :na])
            nc.scalar.copy(osbB[:, bo:bo + nb], psb[c][:, :nb])

        # outs: A region on SP (main + tail), B region on Pool + ACT tail
        aoff = plan["a_off"]
        boff = plan["b_off"]
        a3 = int(aoff[NCH - 1])
        nc.sync.dma_start(y[:, 0:a3], osbA[:, 0:a3])
        nc.sync.dma_start(y[:, a3:int(aoff[-1])], osbA[:, a3:int(aoff[-1])])
        bbase = int(aoff[-1])
        b3 = int(boff[NCH - 1])
        nc.gpsimd.dma_start(y[:, bbase:b3], osbB[:, 0:b3 - bbase])
        nc.scalar.dma_start(y[:, b3:nx], osbB[:, b3 - bbase:nx - bbase])

    _split_multi_waits(nc)
    return nc


def _counts(rbuck):
    return np.bincount(rbuck, minlength=NBUCKET)


_NC_CACHE = {}


def kernel(positions, outputs, table):
    positions = np.asarray(positions)
    outputs = np.asarray(outputs, dtype=np.float32)
    table = np.asarray(table, dtype=np.float32)
    T, B = positions.shape
    n_cores = 8

    rbuck = np.minimum(positions, NBUCKET - 1).astype(np.int64)
    rbuck = rbuck.reshape(n_cores, N_CORE)
    x = outputs.reshape(n_cores, N_CORE, D)
    table_bf = table.astype(BF)

    counts = np.stack([_counts(rbuck[c]) for c in range(n_cores)])
    slot_len = tuple(int(v) for v in counts.max(axis=0))

    key = slot_len
    if key not in _NC_CACHE:
        _NC_CACHE[key] = build_nc(slot_len)
    nc = _NC_CACHE[key]
    _NC_CACHE["nc"] = nc  # for test.py's TimelineSim hook
    plan = _plan(slot_len)
    nx = plan["nx"]
    soff = plan["slot_off"]

    # y column -> x column mapping (y regions are [A0..A3 | B0..B3])
    cuts, ab = plan["cuts"], plan["ab"]
    y2x = np.empty(nx, np.int64)
    pos = 0
    for c in range(NCH):
        y2x[pos:pos + plan["a_len"][c]] = np.arange(cuts[c], ab[c])
        pos += plan["a_len"][c]
    for c in range(NCH):
        y2x[pos:pos + plan["b_len"][c]] = np.arange(ab[c], cuts[c + 1])
        pos += plan["b_len"][c]

    Wblk = np.ascontiguousarray(
        table_bf.transpose(1, 0, 2).reshape(P, NBUCKET * P))

    in_maps = []
    scatter = []
    for c in range(n_cores):
        order = np.argsort(rbuck[c], kind="stable")
        src = np.full(nx, -1, np.int64)   # x column -> original row
        ptr = 0
        for k in range(NBUCKET):
            ck = int(counts[c][k])
            src[soff[k]:soff[k] + ck] = order[ptr:ptr + ck]
            ptr += ck
        xs = np.zeros((nx, D), np.float32)
        valid = src >= 0
        xs[valid] = x[c][src[valid]]
        xT = np.ascontiguousarray(xs.T).astype(BF)
        in_maps.append({"wx": np.concatenate([Wblk, xT], axis=1)})
        scatter.append((src, valid))
    res = run_bass_kernel_spmd(nc, in_maps, list(range(n_cores)))

    out = np.empty((n_cores, N_CORE, D), np.float32)
    for c in range(n_cores):
        yT = np.asarray(res.results[c]["y"]).astype(np.float32)
        src, valid = scatter[c]
        # y col j holds x col y2x[j]
        xsrc = src[y2x]
        ok = xsrc >= 0
        out[c][xsrc[ok]] = yT.T[ok]
    return out.reshape(T, B, D)


# revision 25
# speedup vs baseline: 1.3692x; 1.0753x over previous
"""MultiPositionTransfer kernel for 8 TRN2 NeuronCores (Bass/Tile).

Computes out[t,b,:] = outputs[t,b,:] @ table[min(positions[t,b], 8)] for
positions [512,32] int, outputs [512,32,128] f32, table [9,128,128] f32.
Sharding: data-parallel over T across 8 cores (2048 vectors per core);
the small table is replicated.

Per-core algorithm — host bucket-sort, static slotted matmul:

The host sorts each core's 2048 vectors by bucket k = min(pos, 8) and
packs them as xT [128d, NX] where bucket k's columns live in slot
[O_k, O_k + L_k) with L_k = max over cores of count(k) (JIT-baked into
the cached program; other distributions recompile; pathological ones
recurse on T-halves). The device runs one plain matmul per
(slot x chunk x psum-bank) piece:
    yT[:, piece] = table[k]^T @ xT[:, piece]
No masks, no transposes, no 128-alignment padding of slots; each column
is streamed through the PE exactly once. Slack columns (cores with fewer
than L_k entries) compute garbage the host drops.

Schedule (all I/O bf16, PSUM f32):
- W (the 9-block table) + x arrive in 4 chunked DMAs on the SP ring.
- Dummy matmuls pre-warm the PE p-state ramp and fill inter-chunk gaps
  so real matmuls run at the peak 0.42ns/col rate.
- Per chunk, PSUM is split psA/psB; DVE copies A halves, ACT copies B
  halves into per-engine SBUF regions (no cross-engine tile writes).
- Output y = [A0 | A1 A2 | B0 B1 B2 | C3]: A0 leaves via a plain DMA;
  the rest via SWDGE prepared scatter-adds fired by trigger_dma at
  ~zero issue latency (the 2 zero-fill DMAs ride the SP ring between
  the input chunks, guarded by a semaphore on the triggers).
"""

import numpy as np
from contextlib import ExitStack

import ml_dtypes

import concourse.bass as bass
import concourse.tile as tile
from concourse import mybir
from concourse.bass_utils import run_bass_kernel_spmd
from concourse.instruction_name_ordered_set import InstructionNameOrderedSet
from concourse.vector_clock import ScopedClock, VectorClock

P = 128
D = 128
NBUCKET = 9
F32 = mybir.dt.float32
BF16 = mybir.dt.bfloat16
I16 = mybir.dt.int16
BF = ml_dtypes.bfloat16
PSUM_BANK = 512        # f32 cols per PSUM bank
PREWARM = 30           # dummy matmuls before the real stream (PE ramp-up)
FILLER = 4             # dummy matmuls between chunks (keep PE at peak)
NX_LIMIT = 3584        # PSUM budget guard; above this, recurse on T


def _drain_and_barrier_no_drain_waits(self, tick_clock, wait_clock):
    from concourse.tile_scheduler import PROC_NAMES
    nc = self.nc
    vec = tick_clock.global_clock
    for proc in range(len(vec)):
        if vec[proc] <= 0:
            continue
        # DMASW lanes are ticked by prepare_only scatter preps whose
        # completion sems are our own dsems (waited explicitly on Pool
        # before teardown); the internal DMASW sems never move
        if PROC_NAMES[proc].startswith("DMASW"):
            continue
        unit = VectorClock([vec[p] if p == proc else 0 for p in range(len(vec))])
        nop_inst = nc.sync.nop()
        wait_clock.add_sem_waits(nop_inst.ins, ScopedClock({None: unit}))
    for eng in nc.engines.values():
        eng.drain()
    nc.all_engine_barrier(sem_only=True)
    assert self.sems is not None
    popped = nc._tile_sem_poison_stack.pop()
    assert popped is self._sem_poison
    nc.clear_and_free_semaphores(list(self.sems.allocated().values()))
    nc.all_engine_barrier(sem_only=True)


def _install_tile_compat():
    tile.TileContext._drain_and_barrier = _drain_and_barrier_no_drain_waits


def _split_multi_waits(nc):
    for fn in nc.m.functions:
        for bb in fn.blocks:
            insts = bb.instructions
            for i in range(len(insts) - 1, -1, -1):
                inst = insts[i]
                si = inst.sync_info
                if si is None:
                    continue
                waits = list(si.on_wait)
                cap = 0 if inst.opcode == "Drain" else 1
                if len(waits) <= cap:
                    continue
                keep = waits[len(waits) - cap:] if cap else []
                hoist = waits[: len(waits) - cap] if cap else waits
                nops = []
                for k, w in enumerate(hoist):
                    nops.append(mybir.InstNoOp(
                        name=f"{inst.name}-wsplit{k}",
                        engine=inst.engine,
                        sync_info=mybir.SyncInfo(on_wait=[w], on_update=[]),
                        bass_nofuse=True,
                    ))
                inst.sync_info = mybir.SyncInfo(
                    on_wait=keep, on_update=list(si.on_update))
                insts[i:i] = nops


def _plan(slot_len):
    """Static program plan from per-bucket slot lengths (tuple of 9 ints).

    x layout: slots packed back to back, padded to a 128 multiple (the pad
    extends the last nonempty slot; its columns are computed and dropped).
    Chunks: 3 roughly-even 128-aligned chunks + one 128-col tail chunk C3.
    Chunks 0-2 split ~48/52 into A (DVE-copied) / B (ACT-copied) halves at
    128-aligned points. y layout: [A0 | A1 A2 | B0 B1 B2 | C3].
    """
    slot_len = tuple(int(v) for v in slot_len)
    raw = sum(slot_len)
    nx = -(-raw // P) * P
    pad = nx - raw
    ext = list(slot_len)
    for k in range(NBUCKET - 1, -1, -1):
        if ext[k] > 0:
            ext[k] += pad
            break
    soff = np.concatenate([[0], np.cumsum(ext)])
    assert soff[-1] == nx
    body = nx - P
    c1 = (body // 3) // P * P
    c2 = (2 * body // 3) // P * P
    cuts = [0, c1, c2, body, nx]
    ab = []
    for c in range(3):
        w = cuts[c + 1] - cuts[c]
        ab.append(cuts[c] + (int(0.48 * w) + P - 1) // P * P)
    # y regions in x-column terms
    a_seg = [(cuts[c], ab[c]) for c in range(3)]
    b_seg = [(ab[c], cuts[c + 1]) for c in range(3)]
    c_seg = (body, nx)
    yorder = [a_seg[0], a_seg[1], a_seg[2], b_seg[0], b_seg[1], b_seg[2], c_seg]
    ylen = [e - s for s, e in yorder]
    yoff = np.concatenate([[0], np.cumsum(ylen)])
    return dict(slot_len=slot_len, slot_ext=tuple(ext), slot_off=soff, nx=nx,
                cuts=cuts, ab=ab, a_seg=a_seg, b_seg=b_seg, c_seg=c_seg,
                yorder=yorder, yoff=yoff)


def _pieces(plan, s, e):
    """Matmul pieces (xs, xe, bucket) covering x cols [s, e), split at slot
    boundaries and at PSUM-bank boundaries relative to s (tile-local)."""
    out = []
    soff = plan["slot_off"]
    for k in range(NBUCKET):
        ks, ke = int(soff[k]), int(soff[k + 1])
        lo, hi = max(s, ks), min(e, ke)
        p = lo
        while p < hi:
            bank_end = s + ((p - s) // PSUM_BANK + 1) * PSUM_BANK
            q = min(hi, bank_end)
            out.append((p, q, k))
            p = q
    return out


def build_nc(slot_len):
    _install_tile_compat()
    nc = bass.Bass("TRN2", target_bir_lowering=False, debug=False,
                   num_swdge_queues=1)
    plan = _plan(slot_len)
    nx = plan["nx"]
    nwc = NBUCKET * P
    wx = nc.dram_tensor("wx", [P, nwc + nx], BF16, kind="ExternalInput").ap()
    zro = nc.dram_tensor("zro", [P, nx], BF16, kind="ExternalInput").ap()
    y = nc.dram_tensor("y", [P, nx], BF16, kind="ExternalOutput").ap()

    cuts, ab = plan["cuts"], plan["ab"]
    yoff = plan["yoff"]
    a0_len = int(yoff[1])
    dsems = [nc.alloc_semaphore(f"dsem{i}") for i in range(3)]

    with tile.TileContext(nc) as tc, ExitStack() as ctx:
        const = ctx.enter_context(tc.tile_pool(name="const", bufs=1))
        psp = ctx.enter_context(tc.tile_pool(name="ps", bufs=1, space="PSUM"))

        dummy = const.tile([P, P], BF16, tag="dummy")
        nc.gpsimd.memset(dummy[:], 0)
        idx = const.tile([P, 8], I16, tag="idx")
        nc.gpsimd.iota(idx[:], [[16, 8]], base=0, channel_multiplier=1)

        WX = const.tile([P, nwc + nx], BF16)
        # SP ring: in0(W+x0), in1, in2, z1, in3, z2 — ring order guarantees
        # the zero-fills land after the inputs without extra sems
        nc.sync.dma_start(WX[:, 0:nwc + cuts[1]], wx[:, 0:nwc + cuts[1]])
        nc.sync.dma_start(WX[:, nwc + cuts[1]:nwc + cuts[2]],
                          wx[:, nwc + cuts[1]:nwc + cuts[2]])
        nc.sync.dma_start(WX[:, nwc + cuts[2]:nwc + cuts[3]],
                          wx[:, nwc + cuts[2]:nwc + cuts[3]])
        nc.sync.dma_start(y[:, a0_len:int(yoff[3])],
                          zro[:, a0_len:int(yoff[3])])
        nc.sync.dma_start(WX[:, nwc + cuts[3]:nwc + nx],
                          wx[:, nwc + cuts[3]:nwc + nx])
        nc.sync.dma_start(y[:, int(yoff[3]):nx], zro[:, int(yoff[3]):nx])

        # PSUM tiles: psA/psB for chunks 0-2, psC for the tail chunk; the
        # dummy matmul scratch block is appended to psC's bank
        psa, psb = [], []
        for c in range(3):
            na = ab[c] - cuts[c]
            nb = cuts[c + 1] - ab[c]
            pa = psp.tile([P, na], F32, space="PSUM", tag=f"psa{c}")
            pb = psp.tile([P, nb], F32, space="PSUM", tag=f"psb{c}")
            psa.append(pa)
            psb.append(pb)
        psc = psp.tile([P, 2 * P], F32, space="PSUM", tag="psc")

        def dummy_mm(n):
            for _ in range(n):
                nc.tensor.matmul(psc[:, P:2 * P], dummy[:], dummy[:],
                                 start=True, stop=True)

        def real_mms(ps, s, e):
            for (xs, xe, k) in _pieces(plan, s, e):
                nc.tensor.matmul(ps[:, xs - s:xe - s],
                                 WX[:, k * P:(k + 1) * P],
                                 WX[:, nwc + xs:nwc + xe],
                                 start=True, stop=True)

        # SBUF staging: A0 gets its own tile (plain out mustn't wait the
        # later DVE copies via whole-tile dep tracking); DVE also fills
        # [A1|A2|C3], ACT fills [B0|B1|B2]
        osbA0 = const.tile([P, a0_len], BF16, tag="osbA0")
        osbA = const.tile([P, int(yoff[3]) - int(yoff[1]) + P],
                          BF16, tag="osbA")
        osbB = const.tile([P, int(yoff[6]) - int(yoff[3])], BF16, tag="osbB")

        dummy_mm(PREWARM)
        a_acc = 0
        b_acc = 0
        for c in range(3):
            na = ab[c] - cuts[c]
            nb = cuts[c + 1] - ab[c]
            real_mms(psa[c], cuts[c], ab[c])
            real_mms(psb[c], ab[c], cuts[c + 1])
            dummy_mm(FILLER)
            if c == 0:
                nc.vector.tensor_copy(out=osbA0[:], in_=psa[c][:, :na])
            else:
                nc.vector.tensor_copy(out=osbA[:, a_acc:a_acc + na],
                                      in_=psa[c][:, :na])
                a_acc += na
            nc.scalar.copy(osbB[:, b_acc:b_acc + nb], psb[c][:, :nb])
            b_acc += nb
        real_mms(psc, cuts[3], nx)
        nc.vector.tensor_copy(out=osbA[:, a_acc:a_acc + P], in_=psc[:, :P])

        # outs. A0 plain on the SP ring; A1A2 / B0B1B2 / C3 as prepared
        # scatter-adds (zero-latency trigger after the copies, zero-fill
        # ordering via zsem)
        nc.sync.dma_start(y[:, 0:a0_len], osbA0[:])

        def prep_scatter(src_ap, ycol, ncols, sem):
            return nc.gpsimd.dma_scatter_add(
                y[:, ycol:ycol + ncols],
                src_ap.rearrange("p (a c) -> p a c", a=1),
                idx[:],
                num_idxs=P,
                num_idxs_reg=P,
                elem_size=ncols,
                elem_step=nx,
                prepare_only=True,
                sem=sem,
            )

        prep_scatter(osbA[:, 0:a_acc], a0_len, a_acc, dsems[0])
        prep_scatter(osbB[:, 0:b_acc], int(yoff[3]), b_acc, dsems[1])
        prep_scatter(osbA[:, a_acc:a_acc + P], int(yoff[6]), P, dsems[2])
        # one trigger fires all three ring entries in prep order; tile gives
        # it the full dep set (preps' Pool ticks, the copies via deferred
        # src reads, and the zero-fill DMAs via y-range WAW tracking)
        trig = nc.gpsimd.trigger_dma(count=None)
        # the dsem waits have no tile-visible producer; pin them after the
        # trigger so the scheduler can't hoist them
        tdep = InstructionNameOrderedSet()
        tdep.add(trig.ins.name)
        for ds in dsems:
            w = nc.gpsimd.wait_ge(ds, 16)
            w.ins.add_nosync_dependencies_from(tdep)

    _split_multi_waits(nc)
    return nc


def _counts(rbuck):
    return np.bincount(rbuck, minlength=NBUCKET)


_NC_CACHE = {}


def kernel(positions, outputs, table):
    positions = np.asarray(positions)
    outputs = np.asarray(outputs, dtype=np.float32)
    table = np.asarray(table, dtype=np.float32)
    T, B = positions.shape
    n_cores = 8
    n_core = T * B // n_cores

    rbuck = np.minimum(positions, NBUCKET - 1).astype(np.int64)
    rbuck = rbuck.reshape(n_cores, n_core)
    x = outputs.reshape(n_cores, n_core, D)
    table_bf = table.astype(BF)

    counts = np.stack([_counts(rbuck[c]) for c in range(n_cores)])
    slot_len = tuple(int(v) for v in counts.max(axis=0))

    if sum(slot_len) > NX_LIMIT and T >= 16:
        # pathological distribution: recurse on T halves (device still does
        # all the math; just two smaller launches)
        h = T // 2
        top = kernel(positions[:h], outputs[:h], table)
        bot = kernel(positions[h:], outputs[h:], table)
        return np.concatenate([top, bot], axis=0)

    key = slot_len
    if key not in _NC_CACHE:
        _NC_CACHE[key] = build_nc(slot_len)
    nc = _NC_CACHE[key]
    _NC_CACHE["nc"] = nc  # for test.py's TimelineSim hook
    plan = _plan(slot_len)
    nx = plan["nx"]
    soff = plan["slot_off"]

    # y column j holds x column y2x[j]
    y2x = np.concatenate([np.arange(s, e) for s, e in plan["yorder"]])
    assert len(y2x) == nx

    Wblk = np.ascontiguousarray(
        table_bf.transpose(1, 0, 2).reshape(P, NBUCKET * P))
    zbuf = np.zeros((P, nx), BF)

    in_maps = []
    scatter = []
    for c in range(n_cores):
        order = np.argsort(rbuck[c], kind="stable")
        src = np.full(nx, -1, np.int64)   # x column -> original row
        ptr = 0
        for k in range(NBUCKET):
            ck = int(counts[c][k])
            src[soff[k]:soff[k] + ck] = order[ptr:ptr + ck]
            ptr += ck
        xs = np.zeros((nx, D), np.float32)
        valid = src >= 0
        xs[valid] = x[c][src[valid]]
        xT = np.ascontiguousarray(xs.T).astype(BF)
        in_maps.append({"wx": np.concatenate([Wblk, xT], axis=1), "zro": zbuf})
        scatter.append(src)
    res = run_bass_kernel_spmd(nc, in_maps, list(range(n_cores)))

    out = np.empty((n_cores, n_core, D), np.float32)
    for c in range(n_cores):
        yT = np.asarray(res.results[c]["y"]).astype(np.float32)
        xsrc = scatter[c][y2x]
        ok = xsrc >= 0
        out[c][xsrc[ok]] = yT.T[ok]
    return out.reshape(T, B, D)
